# revision 1
# baseline (speedup 1.0000x reference)
"""GraphWaveNet encoder on 8 Trainium2 NeuronCores (Bass/Tile).

Strategy (graph-level data parallel, hint-aligned):
  - 512 graphs -> 8 cores x 64 graphs. batch_idx is sorted, so each core owns a
    contiguous node range; ranges are padded to a common NPAD.
  - GCN message passing: edges (incl. self-loops) are assigned to the core that
    owns the dst node, sorted by dst block (128 nodes), bucketed by src window
    (32768 rows, int16 dma_gather limit). Gathers pull x/h rows from a
    replicated bf16 table in DRAM via dma_gather; scatter-add is a one-hot
    matmul accumulated in PSUM (norm folded into the one-hot).
  - BN+ReLU folded into per-layer scale/bias on the host (eval-mode BN).
  - h1 is allgathered (bf16) across cores between the two GCN layers.
  - Per-graph mean pool: host-built one-hot (1/cnt) matmul, accumulated in PSUM.
  - Edge encoder: edges assigned by graph(src); relu(ea@We1+be1) scaled by
    1/ecnt per edge (folded into the ACT scale), one-hot matmul into per-graph
    sums, then @We2 + masked be2. Output per core is [128 feat, 64 graphs];
    host transposes and concatenates.
"""

import numpy as np
import ml_dtypes

import concourse.bass as bass
import concourse.bacc as bacc
import concourse.mybir as mybir
import concourse.tile as tile
from concourse.library_config import mlp

# ---- problem constants (fixed by the grading harness) ----
N = 100000
E = 1600000
D = 128
DE = 16
G = 512
EPS = 1e-5
NC = 8
GPC = G // NC          # graphs per core
P = 128
W = 32768              # dma_gather int16 window (rows)

# ---- hardcoded sharding structure (validated against the fixed input seed;
#      margins included, overflow asserts at runtime) ----
NPAD = 12800           # padded nodes per core (max real count ~12634)
NB = NPAD // P         # node blocks per core
NTOT = NC * NPAD       # gather-table rows (102400)
NWIN = (NTOT + W - 1) // W   # 4 src windows
# per-(block,window) gather slot capacities, multiples of 128
CAPS = (896, 896, 896, 256)
TPB = sum(CAPS) // P   # gather tiles per block (26)
SLOTS = sum(CAPS)      # slots per block
TE = 1640              # edge-encoder tiles per core (max real ~1578)

F32 = mybir.dt.float32
BF16 = mybir.dt.bfloat16
I16 = mybir.dt.int16

_cache = {}


def _build_nc():
    nc = bacc.Bacc("TRN2", target_bir_lowering=False, debug=False, num_devices=NC)

    # gather tables (replicated)
    x_table = nc.dram_tensor("x_table", [NTOT, D], BF16, kind="ExternalInput")
    # per-core GCN edge data
    idx16 = nc.dram_tensor("idx16", [NB, P, SLOTS // 16], I16, kind="ExternalInput")
    meta = nc.dram_tensor("meta", [NB, P, 2 * TPB], F32, kind="ExternalInput")
    pool_oh = nc.dram_tensor("pool_oh", [NB, P, GPC], BF16, kind="ExternalInput")
    # per-core edge-encoder data
    ea_t = nc.dram_tensor("ea_t", [DE, TE * P], BF16, kind="ExternalInput")
    emeta = nc.dram_tensor("emeta", [TE // 8, P, 16], F32, kind="ExternalInput")
    # small replicated weights
    w1f = nc.dram_tensor("w1f", [D, D], F32, kind="ExternalInput")
    t1r = nc.dram_tensor("t1r", [1, D], F32, kind="ExternalInput")
    w2f = nc.dram_tensor("w2f", [D, D], F32, kind="ExternalInput")
    t2r = nc.dram_tensor("t2r", [1, D], F32, kind="ExternalInput")
    we1 = nc.dram_tensor("we1", [DE, D], BF16, kind="ExternalInput")
    be1 = nc.dram_tensor("be1", [1, D], BF16, kind="ExternalInput")
    we2 = nc.dram_tensor("we2", [D, D], BF16, kind="ExternalInput")
    be2c = nc.dram_tensor("be2c", [1, D], BF16, kind="ExternalInput")
    emask = nc.dram_tensor("emask", [1, GPC], BF16, kind="ExternalInput")
    iota128 = nc.dram_tensor("iota128", [P, P], F32, kind="ExternalInput")
    iota64 = nc.dram_tensor("iota64", [P, GPC], F32, kind="ExternalInput")

    out_t = nc.dram_tensor("out_t", [D, GPC], F32, kind="ExternalOutput")

    # internal DRAM
    h1_local = nc.dram_tensor("h1_local", [NPAD, D], BF16)
    h1_all = nc.dram_tensor("h1_all", [NTOT, D], BF16, addr_space="Shared")

    with tile.TileContext(nc) as tc:
        with (
            tc.tile_pool(name="const", bufs=1) as cpool,
            tc.tile_pool(name="sb", bufs=3) as pool,
            tc.tile_pool(name="ps", bufs=2, space="PSUM") as psum,
            tc.tile_pool(name="ps1", bufs=1, space="PSUM") as psum1,
            tc.tile_pool(name="acc", bufs=1, space="PSUM") as psacc,
        ):
            nc.gpsimd.load_library(mlp)

            # constants
            io128 = cpool.tile([P, P], F32)
            nc.sync.dma_start(out=io128[:], in_=iota128[:])
            io64 = cpool.tile([P, GPC], F32)
            nc.sync.dma_start(out=io64[:], in_=iota64[:])
            ones1 = cpool.tile([1, P], F32)
            nc.vector.memset(ones1[:], 1.0)
            cw1 = cpool.tile([D, D], F32)
            nc.sync.dma_start(out=cw1[:], in_=w1f[:])
            ct1 = cpool.tile([1, D], F32)
            nc.sync.dma_start(out=ct1[:], in_=t1r[:])
            cw2 = cpool.tile([D, D], F32)
            nc.sync.dma_start(out=cw2[:], in_=w2f[:])
            ct2 = cpool.tile([1, D], F32)
            nc.sync.dma_start(out=ct2[:], in_=t2r[:])
            cwe1 = cpool.tile([DE, D], BF16)
            nc.sync.dma_start(out=cwe1[:], in_=we1[:])
            cbe1 = cpool.tile([1, D], BF16)
            nc.sync.dma_start(out=cbe1[:], in_=be1[:])
            cwe2 = cpool.tile([D, D], BF16)
            nc.sync.dma_start(out=cwe2[:], in_=we2[:])
            cbe2 = cpool.tile([1, D], BF16)
            nc.sync.dma_start(out=cbe2[:], in_=be2c[:])
            cmask = cpool.tile([1, GPC], BF16)
            nc.sync.dma_start(out=cmask[:], in_=emask[:])
            ones1b = cpool.tile([1, P], BF16)
            nc.vector.memset(ones1b[:], 1.0)

            pool_ps = psacc.tile([D, GPC], F32, space="PSUM")
            es_ps = psacc.tile([D, GPC], F32, space="PSUM")

            def gcn_layer(table_ap, cw, ct, layer):
                last = layer == 2
                for b in range(NB):
                    ix = pool.tile([P, SLOTS // 16], I16, tag="ix")
                    nc.sync.dma_start(out=ix[:], in_=idx16[b])
                    mt = pool.tile([P, 2 * TPB], F32, tag="mt")
                    nc.sync.dma_start(out=mt[:], in_=meta[b])
                    g = pool.tile([P, SLOTS], BF16, tag="g")
                    slot0 = 0
                    for w in range(NWIN):
                        cap = CAPS[w]
                        wbase = w * W
                        wrows = min(W, NTOT - wbase)
                        nc.gpsimd.dma_gather(
                            g[:, slot0:slot0 + cap].rearrange(
                                "p (c d) -> p c d", d=D),
                            table_ap[wbase:wbase + wrows],
                            ix[:, slot0 // 16:(slot0 + cap) // 16],
                            num_idxs=cap, num_idxs_reg=cap,
                            elem_size=D, single_packet=False,
                        )
                        slot0 += cap
                    agg_ps = psum.tile([D, P], F32, space="PSUM", tag="agg")
                    for j in range(TPB):
                        oh = pool.tile([P, P], BF16, tag="oh")
                        nc.vector.tensor_scalar(
                            out=oh[:], in0=io128[:],
                            scalar1=mt[:, 2 * j:2 * j + 1],
                            scalar2=mt[:, 2 * j + 1:2 * j + 2],
                            op0=mybir.AluOpType.is_equal,
                            op1=mybir.AluOpType.mult,
                        )
                        nc.tensor.matmul(
                            out=agg_ps[:], lhsT=g[:, j * P:(j + 1) * P],
                            rhs=oh[:], start=(j == 0), stop=(j == TPB - 1),
                        )
                    agg = pool.tile([D, P], F32, tag="aggs")
                    nc.scalar.copy(out=agg[:], in_=agg_ps[:])
                    h_ps = psum1.tile([P, D], F32, space="PSUM", tag="h")
                    nc.tensor.matmul(out=h_ps[:], lhsT=agg[:], rhs=cw[:],
                                     start=True, stop=False)
                    nc.tensor.matmul(out=h_ps[:], lhsT=ones1[:], rhs=ct[:],
                                     start=False, stop=True)
                    if not last:
                        h = pool.tile([P, D], BF16, tag="h1s")
                        nc.scalar.activation(
                            out=h[:], in_=h_ps[:],
                            func=mybir.ActivationFunctionType.Relu)
                        nc.sync.dma_start(
                            out=h1_local[b * P:(b + 1) * P, :], in_=h[:])
                    else:
                        h = pool.tile([P, D], BF16, tag="h2s")
                        nc.scalar.activation(
                            out=h[:], in_=h_ps[:],
                            func=mybir.ActivationFunctionType.Relu)
                        po = pool.tile([P, GPC], BF16, tag="po")
                        nc.sync.dma_start(out=po[:], in_=pool_oh[b])
                        nc.tensor.matmul(out=pool_ps[:], lhsT=h[:], rhs=po[:],
                                         start=(b == 0), stop=(b == NB - 1))

            gcn_layer(x_table[:], cw1, ct1, 1)

            nc.gpsimd.collective_compute(
                "AllGather", mybir.AluOpType.bypass,
                replica_groups=[list(range(NC))],
                ins=[h1_local[:]], outs=[h1_all[:]],
            )

            gcn_layer(h1_all[:], cw2, ct2, 2)

            # ---- edge encoder ----
            EB = 8  # tiles per metadata batch
            for t in range(TE):
                if t % EB == 0:
                    em = pool.tile([P, 2 * EB], F32, tag="em")
                    nc.sync.dma_start(out=em[:], in_=emeta[t // EB])
                ea = pool.tile([DE, P], BF16, tag="ea")
                nc.sync.dma_start(out=ea[:], in_=ea_t[:, t * P:(t + 1) * P])
                e_ps = psum.tile([P, D], F32, space="PSUM", tag="eps")
                nc.tensor.matmul(out=e_ps[:], lhsT=ea[:], rhs=cwe1[:],
                                 start=True, stop=False)
                nc.tensor.matmul(out=e_ps[:], lhsT=ones1b[:], rhs=cbe1[:],
                                 start=False, stop=True)
                he = pool.tile([P, D], BF16, tag="he")
                j = t % EB
                nc.scalar.activation(
                    out=he[:], in_=e_ps[:],
                    func=mybir.ActivationFunctionType.Relu,
                    scale=em[:, 2 * j + 1:2 * j + 2])
                ohe = pool.tile([P, GPC], BF16, tag="ohe")
                nc.vector.tensor_scalar(
                    out=ohe[:], in0=io64[:],
                    scalar1=em[:, 2 * j:2 * j + 1], scalar2=None,
                    op0=mybir.AluOpType.is_equal,
                )
                nc.tensor.matmul(out=es_ps[:], lhsT=he[:], rhs=ohe[:],
                                 start=(t == 0), stop=(t == TE - 1))

            # ---- finalize: out_t = pool_ps + We2.T@es + be2*mask ----
            es_sb = pool.tile([D, GPC], BF16, tag="essb")
            nc.scalar.copy(out=es_sb[:], in_=es_ps[:])
            er_ps = psum1.tile([D, GPC], F32, space="PSUM", tag="h")
            nc.tensor.matmul(out=er_ps[:], lhsT=cwe2[:], rhs=es_sb[:],
                             start=True, stop=False)
            nc.tensor.matmul(out=er_ps[:], lhsT=cbe2[:], rhs=cmask[:],
                             start=False, stop=True)
            pl = pool.tile([D, GPC], F32, tag="pl")
            nc.vector.tensor_copy(out=pl[:], in_=pool_ps[:])
            fin = pool.tile([D, GPC], F32, tag="fin")
            nc.vector.tensor_tensor(out=fin[:], in0=pl[:], in1=er_ps[:],
                                    op=mybir.AluOpType.add)
            nc.sync.dma_start(out=out_t[:], in_=fin[:])

    nc.compile()
    return nc


def _pack_idx(vals):
    """[n] int16 -> [128, n//16] wrapped in 16 partitions, replicated x8."""
    n = vals.shape[0]
    t = vals.reshape(n // 16, 16).T.astype(np.int16)
    return np.tile(t, (8, 1))


def _preprocess(x, edge_index, batch_idx, edge_attr,
                W1, b1, g1, bt1, m1, v1, W2, b2, g2, bt2, m2, v2,
                We1, be1, We2, be2):
    batch = np.asarray(batch_idx).astype(np.int64)
    src = np.asarray(edge_index[0]).astype(np.int64)
    dst = np.asarray(edge_index[1]).astype(np.int64)
    x = np.asarray(x, dtype=np.float32)
    ea = np.asarray(edge_attr, dtype=np.float32)

    node_core = batch // GPC
    core_start = np.searchsorted(batch, np.arange(NC) * GPC)
    counts = np.append(core_start[1:], N) - core_start
    assert counts.max() <= NPAD, counts.max()
    pid = np.arange(N) - core_start[node_core] + node_core * NPAD

    x_table = np.zeros((NTOT, D), dtype=ml_dtypes.bfloat16)
    x_table[pid] = x.astype(ml_dtypes.bfloat16)

    deg = np.bincount(dst, minlength=N).astype(np.float32) + 1.0
    dinv = 1.0 / np.sqrt(deg)

    es = np.concatenate([src, np.arange(N)])
    ed = np.concatenate([dst, np.arange(N)])
    enorm = np.concatenate([dinv[src] * dinv[dst], dinv * dinv]).astype(np.float32)

    ps, pd = pid[es], pid[ed]
    core_e = pd // NPAD
    local_d = pd % NPAD
    blk = local_d // P
    drel = (local_d % P).astype(np.float32)
    win = ps // W
    iwin = (ps % W).astype(np.int16)

    order = np.lexsort((win, blk, core_e))
    core_e, blk, win = core_e[order], blk[order], win[order]
    drel, iwin, enorm_s = drel[order], iwin[order], enorm[order]

    key = (core_e * NB + blk) * NWIN + win
    bounds = np.searchsorted(key, np.arange(NC * NB * NWIN + 1))

    idx16 = np.zeros((NC, NB, P, SLOTS // 16), np.int16)
    meta = np.zeros((NC, NB, P, 2 * TPB), np.float32)
    meta[..., 0::2] = -1.0
    slot_base = np.concatenate([[0], np.cumsum(CAPS)])
    for c in range(NC):
        for b in range(NB):
            kb = (c * NB + b) * NWIN
            vals = np.zeros(SLOTS, np.int16)
            dr = np.full(SLOTS, -1.0, np.float32)
            nm = np.zeros(SLOTS, np.float32)
            for w in range(NWIN):
                s0, s1 = bounds[kb + w], bounds[kb + w + 1]
                cnt = s1 - s0
                assert cnt <= CAPS[w], (c, b, w, cnt)
                sb = slot_base[w]
                vals[sb:sb + cnt] = iwin[s0:s1]
                dr[sb:sb + cnt] = drel[s0:s1]
                nm[sb:sb + cnt] = enorm_s[s0:s1]
            idx16[c, b] = _pack_idx(vals)
            sl = np.arange(SLOTS)
            meta[c, b, sl % P, 2 * (sl // P)] = dr
            meta[c, b, sl % P, 2 * (sl // P) + 1] = nm

    # pooling one-hot
    gcnt = np.bincount(batch, minlength=G).astype(np.float32)
    pool_oh = np.zeros((NC, NB, P, GPC), np.float32)
    loc = np.arange(N) - core_start[node_core]
    gl = batch - node_core * GPC
    pool_oh[node_core, loc // P, loc % P, gl] = 1.0 / np.maximum(gcnt[batch], 1.0)
    pool_oh = pool_oh.astype(ml_dtypes.bfloat16)

    # edge encoder
    egraph = batch[src]
    ecore = egraph // GPC
    ecnt = np.bincount(egraph, minlength=G).astype(np.float32)
    eorder = np.argsort(ecore, kind="stable")
    ecore_s, egr_s = ecore[eorder], egraph[eorder]
    ebounds = np.searchsorted(ecore_s, np.arange(NC + 1))
    assert (np.diff(ebounds) <= TE * P).all(), np.diff(ebounds).max()
    ea_t = np.zeros((NC, DE, TE * P), np.float32)
    emeta = np.zeros((NC, TE, P, 2), np.float32)
    emeta[..., 0] = -1.0
    for c in range(NC):
        s0, s1 = ebounds[c], ebounds[c + 1]
        cnt = s1 - s0
        sel = eorder[s0:s1]
        ea_t[c, :, :cnt] = ea[sel].T
        fl = np.arange(cnt)
        emeta[c, fl // P, fl % P, 0] = (egr_s[s0:s1] - c * GPC).astype(np.float32)
        emeta[c, fl // P, fl % P, 1] = 1.0 / np.maximum(ecnt[egr_s[s0:s1]], 1.0)
    ea_t = ea_t.astype(ml_dtypes.bfloat16)
    emask = (ecnt.reshape(NC, GPC) > 0).astype(ml_dtypes.bfloat16)[:, None, :]

    # folded weights
    def fold(Wm, bm, gm, btm, mm, vm):
        s = (gm / np.sqrt(vm + EPS)).astype(np.float32)
        return (np.asarray(Wm, np.float32) * s[None, :],
                ((np.asarray(bm, np.float32) - mm) * s + btm).astype(np.float32))

    w1f, t1 = fold(W1, b1, g1, bt1, m1, v1)
    w2f, t2 = fold(W2, b2, g2, bt2, m2, v2)

    common = {
        "x_table": x_table,
        "w1f": w1f, "t1r": t1[None, :].astype(np.float32),
        "w2f": w2f, "t2r": t2[None, :].astype(np.float32),
        "we1": np.asarray(We1, np.float32).astype(ml_dtypes.bfloat16),
        "be1": np.asarray(be1, np.float32)[None, :].astype(ml_dtypes.bfloat16),
        "we2": np.asarray(We2, np.float32).astype(ml_dtypes.bfloat16),
        "be2c": np.asarray(be2, np.float32)[None, :].astype(ml_dtypes.bfloat16),
        "iota128": np.tile(np.arange(P, dtype=np.float32)[None, :], (P, 1)),
        "iota64": np.tile(np.arange(GPC, dtype=np.float32)[None, :], (P, 1)),
    }
    in_maps = []
    for c in range(NC):
        m = dict(common)
        m["idx16"] = idx16[c]
        m["meta"] = meta[c]
        m["pool_oh"] = pool_oh[c]
        m["ea_t"] = ea_t[c]
        m["emeta"] = emeta[c].reshape(TE // 8, 8, P, 2).transpose(0, 2, 1, 3).reshape(TE // 8, P, 16)
        m["emask"] = emask[c]
        in_maps.append(m)
    return in_maps


def _get_runner():
    if "runner" not in _cache:
        nc = _build_nc()
        from concourse.bass_utils import run_bass_kernel_spmd
        _cache["nc"] = nc
        _cache["run"] = lambda in_maps: run_bass_kernel_spmd(
            nc, in_maps, list(range(NC)))
    return _cache["nc"], _cache["run"]


def kernel(x, edge_index, batch_idx, edge_attr, num_graphs,
           W1, b1, g1, bt1, m1, v1, W2, b2, g2, bt2, m2, v2,
           We1, be1, We2, be2):
    in_maps = _preprocess(x, edge_index, batch_idx, edge_attr,
                          W1, b1, g1, bt1, m1, v1, W2, b2, g2, bt2, m2, v2,
                          We1, be1, We2, be2)
    nc, run = _get_runner()
    res = run(in_maps)
    out = np.zeros((G, D), np.float32)
    for c in range(NC):
        out[c * GPC:(c + 1) * GPC, :] = res.results[c]["out_t"].T
    return out



# revision 9
# speedup vs baseline: 14535.3236x; 14535.3236x over previous
"""GraphWaveNet encoder on 8 Trainium2 NeuronCores (Bass/Tile) — v2.

Graph-level data parallel (512 graphs -> 8 cores x 64). batch_idx sorted, so
each core owns a contiguous node range padded to NPAD. Per layer, GCN
aggregation gathers source rows (bf16) from a replicated DRAM table with
dma_gather, scatters them into per-block [feat,dst] PSUM accumulators with
one-hot matmuls (norm folded into the one-hot), then applies the folded W/BN
and ReLU.

Structure (v2):
  - gathers batched per 8-block group (4 window calls + 1 linear self-row DMA)
  - per-(block,window) slot capacities = max over the 8 cores (SPMD-uniform),
    rounded to 128; padding gathers window row 0 and is zeroed by the one-hot
  - one-hot tiles via per-tile tensor_scalar is_equal+mult (bf16 -> DVE 4x)
  - self-loops excluded from the gather: linear DMA + diag one-hot tile
  - edge encoder transposed: We1 variants stationary (K=64, bases 0/64),
    streams 512-edge graph-aligned chunks, ReLU + per-chunk sum fused into
    scalar.activation(bias=be1, accum_out=...); We2/be2 applied on the host
  - per-graph pooling via host-built one-hot matmul accumulated in PSUM
"""

import numpy as np
import ml_dtypes

import concourse.bass as bass
import concourse.bacc as bacc
import concourse.mybir as mybir
import concourse.tile as tile
from concourse.library_config import mlp

# ---- problem constants (fixed by the grading harness) ----
N = 100000
E = 1600000
D = 128
DE = 16
G = 512
EPS = 1e-5
NC = 8
GPC = G // NC          # graphs per core
P = 128
W = 32768              # dma_gather int16 window (rows)

NPAD = 13312           # padded nodes per core (multiple of 1024)
NB = NPAD // P         # node blocks per core (104)
GPG = 8                # blocks per gather group
NG = NB // GPG         # groups per core (13)
NTOT = NC * NPAD       # gather-table rows (106496)
NWIN = (NTOT + W - 1) // W   # 4 src windows
ECH_CHUNK = 512        # edges per edge-encoder chunk

F32 = mybir.dt.float32
BF16 = mybir.dt.bfloat16
I16 = mybir.dt.int16

_cache = {}


class _Layout:
    """Group slot layout shared by _build_nc and _preprocess.

    caps_bw: [NB, NWIN] per-(block,window) slot capacity (multiples of 128).
    Per group: [w0: b0..b7 | w1: ... | w3: ... | self: 8*128] slot regions.
    """

    def __init__(self, caps_bw):
        self.caps_bw = np.asarray(caps_bw, np.int64)
        assert self.caps_bw.shape == (NB, NWIN)
        self.wstart = np.zeros((NG, NWIN), np.int64)   # window region start
        self.wsize = np.zeros((NG, NWIN), np.int64)
        self.bstart = np.zeros((NB, NWIN), np.int64)   # block start (in-group)
        self.selfb = np.zeros(NG, np.int64)
        self.gslots = np.zeros(NG, np.int64)
        self.tiles = [[] for _ in range(NB)]           # per block: g offsets
        self.mstart = np.zeros(NB, np.int64)           # meta col start (in-grp)
        self.tpb = np.zeros(NB, np.int64)
        self.mcols = np.zeros(NG, np.int64)
        for grp in range(NG):
            off = 0
            for w in range(NWIN):
                self.wstart[grp, w] = off
                for b in range(GPG):
                    B = grp * GPG + b
                    self.bstart[B, w] = off
                    off += self.caps_bw[B, w]
                self.wsize[grp, w] = off - self.wstart[grp, w]
            self.selfb[grp] = off
            self.gslots[grp] = off + GPG * P
            mc = 0
            for b in range(GPG):
                B = grp * GPG + b
                t = []
                for w in range(NWIN):
                    for k in range(self.caps_bw[B, w] // P):
                        t.append(int(self.bstart[B, w] + k * P))
                t.append(int(self.selfb[grp] + b * P))
                self.tiles[B] = t
                self.mstart[B] = mc
                self.tpb[B] = len(t)
                mc += 2 * len(t)
            self.mcols[grp] = mc
        self.SGMAX = int(self.selfb.max())
        self.GSMAX = int(self.gslots.max())
        self.MCMAX = int(self.mcols.max())
        self.TPBMAX = int(self.tpb.max())


def _build_nc(caps_key, ECH):
    lay = _Layout(np.asarray(caps_key, np.int64).reshape(NB, NWIN))
    ECOLS = ((ECH + 7) // 8) * ECH_CHUNK

    nc = bacc.Bacc("TRN2", target_bir_lowering=False, debug=False, num_devices=NC)

    x_table = nc.dram_tensor("x_table", [NTOT, D], BF16, kind="ExternalInput")
    idx16 = nc.dram_tensor("idx16", [NG, P, lay.SGMAX // 16], I16,
                           kind="ExternalInput")
    meta = nc.dram_tensor("meta", [NG, P, lay.MCMAX], F32, kind="ExternalInput")
    pool_oh = nc.dram_tensor("pool_oh", [NG, P, GPG * GPC], BF16,
                             kind="ExternalInput")
    ea_pack = nc.dram_tensor("ea_pack", [P, ECOLS], BF16, kind="ExternalInput")
    w1f = nc.dram_tensor("w1f", [D, D], BF16, kind="ExternalInput")
    t1r = nc.dram_tensor("t1r", [1, D], BF16, kind="ExternalInput")
    w2f = nc.dram_tensor("w2f", [D, D], BF16, kind="ExternalInput")
    t2r = nc.dram_tensor("t2r", [1, D], BF16, kind="ExternalInput")
    wvar = nc.dram_tensor("wvar", [P, 4 * D], BF16, kind="ExternalInput")
    be1c = nc.dram_tensor("be1c", [D, 1], F32, kind="ExternalInput")
    iota128 = nc.dram_tensor("iota128", [P, P], BF16, kind="ExternalInput")

    out_pool = nc.dram_tensor("out_pool", [D, GPC], F32, kind="ExternalOutput")
    out_eacc = nc.dram_tensor("out_eacc", [D, ECH], F32, kind="ExternalOutput")

    h1_local = nc.dram_tensor("h1_local", [NPAD, D], BF16)
    h1_all = nc.dram_tensor("h1_all", [NTOT, D], BF16, addr_space="Shared")

    with tile.TileContext(nc) as tc:
        with (
            tc.tile_pool(name="const", bufs=1) as cpool,
            tc.tile_pool(name="gp", bufs=2) as gpool,
            tc.tile_pool(name="sb", bufs=3) as pool,
            tc.tile_pool(name="ea", bufs=2) as eapool,
            tc.tile_pool(name="ps", bufs=2, space="PSUM") as psum,
            tc.tile_pool(name="eps", bufs=2, space="PSUM") as epsum,
            tc.tile_pool(name="acc", bufs=1, space="PSUM") as psacc,
        ):
            nc.gpsimd.load_library(mlp)

            io128 = cpool.tile([P, P], BF16)
            nc.sync.dma_start(out=io128[:], in_=iota128[:])
            cw1 = cpool.tile([D, D], BF16)
            nc.sync.dma_start(out=cw1[:], in_=w1f[:])
            ct1 = cpool.tile([1, D], BF16)
            nc.sync.dma_start(out=ct1[:], in_=t1r[:])
            cw2 = cpool.tile([D, D], BF16)
            nc.sync.dma_start(out=cw2[:], in_=w2f[:])
            ct2 = cpool.tile([1, D], BF16)
            nc.sync.dma_start(out=ct2[:], in_=t2r[:])
            cwv = cpool.tile([P, 4 * D], BF16)
            nc.sync.dma_start(out=cwv[:], in_=wvar[:])
            cbe1 = cpool.tile([D, 1], F32)
            nc.sync.dma_start(out=cbe1[:], in_=be1c[:])
            ones1 = cpool.tile([1, P], BF16)
            nc.vector.memset(ones1[:], 1.0)

            pool_ps = psacc.tile([D, GPC], F32, space="PSUM")

            def gcn_layer(table_ap, cw, ct, layer):
                last = layer == 2
                for grp in range(NG):
                    selfb = int(lay.selfb[grp])
                    ix = gpool.tile([P, lay.SGMAX // 16], I16, tag="ix")
                    nc.sync.dma_start(out=ix[:, :selfb // 16],
                                      in_=idx16[grp][:, :selfb // 16])
                    mc = int(lay.mcols[grp])
                    mt = gpool.tile([P, lay.MCMAX], F32, tag="mt")
                    nc.sync.dma_start(out=mt[:, :mc], in_=meta[grp][:, :mc])
                    g = gpool.tile([P, lay.GSMAX], BF16, tag="g")
                    for w in range(NWIN):
                        nwi = int(lay.wsize[grp, w])
                        if nwi == 0:
                            continue
                        s0 = int(lay.wstart[grp, w])
                        wb = w * W
                        wrows = min(W, NTOT - wb)
                        nc.gpsimd.dma_gather(
                            g[:, s0:s0 + nwi].rearrange("p (c d) -> p c d", d=D),
                            table_ap[wb:wb + wrows],
                            ix[:, s0 // 16:(s0 + nwi) // 16],
                            num_idxs=nwi, num_idxs_reg=nwi,
                            elem_size=D, single_packet=False,
                        )
                    # self rows: linear strided DMA
                    nc.sync.dma_start(
                        out=g[:, selfb:selfb + GPG * P].rearrange(
                            "p (c d) -> p c d", d=D),
                        in_=table_ap[grp * GPG * P:(grp + 1) * GPG * P].rearrange(
                            "(c p) d -> p c d", p=P),
                    )
                    if last:
                        po = gpool.tile([P, GPG * GPC], BF16, tag="po")
                        nc.sync.dma_start(out=po[:], in_=pool_oh[grp])
                    else:
                        hs = gpool.tile([P, GPG * D], BF16, tag="hs")
                    for b in range(GPG):
                        B = grp * GPG + b
                        tpb = int(lay.tpb[B])
                        mb = int(lay.mstart[B])
                        # per-tile is_equal+mult: bf16 in/out -> DVE 4x mode
                        oh = pool.tile([P, lay.TPBMAX * P], BF16, tag="oh")
                        for j in range(tpb):
                            nc.vector.tensor_scalar(
                                out=oh[:, j * P:(j + 1) * P], in0=io128[:],
                                scalar1=mt[:, mb + 2 * j:mb + 2 * j + 1],
                                scalar2=mt[:, mb + 2 * j + 1:mb + 2 * j + 2],
                                op0=mybir.AluOpType.is_equal,
                                op1=mybir.AluOpType.mult,
                            )
                        agg_ps = psum.tile([D, P], F32, space="PSUM", tag="agg")
                        for j, o in enumerate(lay.tiles[B]):
                            nc.tensor.matmul(
                                out=agg_ps[:], lhsT=g[:, o:o + P],
                                rhs=oh[:, j * P:(j + 1) * P],
                                start=(j == 0), stop=(j == tpb - 1),
                            )
                        agg = pool.tile([D, P], BF16, tag="aggs")
                        nc.vector.tensor_copy(out=agg[:], in_=agg_ps[:])
                        h_ps = psum.tile([P, D], F32, space="PSUM", tag="h")
                        nc.tensor.matmul(out=h_ps[:], lhsT=agg[:], rhs=cw[:],
                                         start=True, stop=False)
                        nc.tensor.matmul(out=h_ps[:], lhsT=ones1[:], rhs=ct[:],
                                         start=False, stop=True)
                        if not last:
                            nc.scalar.activation(
                                out=hs[:, b * D:(b + 1) * D], in_=h_ps[:],
                                func=mybir.ActivationFunctionType.Relu)
                        else:
                            h2 = pool.tile([P, D], BF16, tag="h2s")
                            nc.scalar.activation(
                                out=h2[:], in_=h_ps[:],
                                func=mybir.ActivationFunctionType.Relu)
                            nc.tensor.matmul(
                                out=pool_ps[:], lhsT=h2[:],
                                rhs=po[:, b * GPC:(b + 1) * GPC],
                                start=(grp == 0 and b == 0),
                                stop=(grp == NG - 1 and b == GPG - 1),
                            )
                    if not last:
                        nc.sync.dma_start(
                            out=h1_local[grp * GPG * P:(grp + 1) * GPG * P]
                                .rearrange("(c p) d -> p c d", p=P),
                            in_=hs[:].rearrange("p (c d) -> p c d", d=D),
                        )

            gcn_layer(x_table[:], cw1, ct1, 1)

            nc.gpsimd.collective_compute(
                "AllGather", mybir.AluOpType.bypass,
                replica_groups=[list(range(NC))],
                ins=[h1_local[:]], outs=[h1_all[:]],
            )

            # ---- edge encoder (overlaps the collective) ----
            eacc = cpool.tile([D, ECH], F32)
            EALOAD = 8 * ECH_CHUNK   # ea columns per load = 64 chunks
            n_loads = (ECOLS + EALOAD - 1) // EALOAD
            ea_tiles = []
            for li in range(n_loads):
                c0 = li * EALOAD
                cw_ = min(EALOAD, ECOLS - c0)
                eat = eapool.tile([P, EALOAD], BF16, tag="eat")
                nc.sync.dma_start(out=eat[:, :cw_], in_=ea_pack[:, c0:c0 + cw_])
                ea_tiles.append(eat)
            for c in range(ECH):
                half = (c % 8) // 4
                j = c % 4
                base = 64 * half
                col = (c // 8) * ECH_CHUNK
                eat = ea_tiles[col // EALOAD]
                lcol = col % EALOAD
                e_ps = epsum.tile([P, ECH_CHUNK], F32, space="PSUM", tag="eps")
                nc.tensor.matmul(
                    out=e_ps[:],
                    lhsT=cwv[base:base + 64, j * D:(j + 1) * D],
                    rhs=eat[base:base + 64, lcol:lcol + ECH_CHUNK],
                    start=True, stop=True)
                dead = pool.tile([P, ECH_CHUNK], BF16, tag="dead")
                nc.scalar.activation(
                    out=dead[:], in_=e_ps[:],
                    func=mybir.ActivationFunctionType.Relu,
                    bias=cbe1[:],
                    accum_out=eacc[:, c:c + 1])
            nc.sync.dma_start(out=out_eacc[:], in_=eacc[:])

            gcn_layer(h1_all[:], cw2, ct2, 2)

            pl = pool.tile([D, GPC], F32, tag="pl")
            nc.vector.tensor_copy(out=pl[:], in_=pool_ps[:])
            nc.sync.dma_start(out=out_pool[:], in_=pl[:])

    nc.compile()
    return nc


def _pack_idx(vals):
    """[n] int16 -> [128, n//16] wrapped in 16 partitions, replicated x8."""
    n = vals.shape[0]
    t = vals.reshape(n // 16, 16).T.astype(np.int16)
    return np.tile(t, (8, 1))


def _preprocess(x, edge_index, batch_idx, edge_attr,
                W1, b1, g1, bt1, m1, v1, W2, b2, g2, bt2, m2, v2,
                We1, be1, We2, be2):
    batch = np.asarray(batch_idx).astype(np.int64)
    src = np.asarray(edge_index[0]).astype(np.int64)
    dst = np.asarray(edge_index[1]).astype(np.int64)
    x = np.asarray(x, dtype=np.float32)
    ea = np.asarray(edge_attr, dtype=np.float32)

    node_core = batch // GPC
    core_start = np.searchsorted(batch, np.arange(NC) * GPC)
    counts = np.append(core_start[1:], N) - core_start
    assert counts.max() <= NPAD, counts.max()
    pid = np.arange(N) - core_start[node_core] + node_core * NPAD

    x_table = np.zeros((NTOT, D), dtype=ml_dtypes.bfloat16)
    x_table[pid] = x.astype(ml_dtypes.bfloat16)

    deg = np.bincount(dst, minlength=N).astype(np.float32) + 1.0
    dinv = 1.0 / np.sqrt(deg)
    dinv_t = np.zeros((NTOT,), np.float32)
    dinv_t[pid] = dinv

    enorm = (dinv[src] * dinv[dst]).astype(np.float32)
    ps, pd = pid[src], pid[dst]
    core_e = pd // NPAD
    blk = (pd % NPAD) // P
    drel = (pd % P).astype(np.float32)
    win = ps // W
    iwin = (ps % W).astype(np.int16)

    order = np.lexsort((win, blk, core_e))
    core_s, blk_s, win_s = core_e[order], blk[order], win[order]
    drel_s, iwin_s, enorm_s = drel[order], iwin[order], enorm[order]

    key = (core_s * NB + blk_s) * NWIN + win_s
    bounds = np.searchsorted(key, np.arange(NC * NB * NWIN + 1))
    cnt = np.diff(bounds).reshape(NC, NB, NWIN)
    caps_bw = (np.ceil(cnt.max(axis=0) / P) * P).astype(np.int64)  # [NB, NWIN]
    lay = _Layout(caps_bw)

    # global group-slot index of each sorted edge
    rank = np.arange(len(order)) - bounds[key]
    slot_in_grp = lay.bstart[blk_s, win_s] + rank
    grp_s = blk_s // GPG

    idx_all = np.zeros((NC, NG, lay.SGMAX), np.int16)
    drel_all = np.full((NC, NG, lay.GSMAX), -1.0, np.float32)
    norm_all = np.zeros((NC, NG, lay.GSMAX), np.float32)
    idx_all[core_s, grp_s, slot_in_grp] = iwin_s
    drel_all[core_s, grp_s, slot_in_grp] = drel_s
    norm_all[core_s, grp_s, slot_in_grp] = enorm_s
    # self region
    sp = np.arange(GPG * P)
    for c in range(NC):
        for grp in range(NG):
            rows = c * NPAD + grp * GPG * P + sp
            sb = int(lay.selfb[grp])
            drel_all[c, grp, sb + sp] = sp % P
            norm_all[c, grp, sb + sp] = dinv_t[rows] ** 2

    idx16 = np.zeros((NC, NG, P, lay.SGMAX // 16), np.int16)
    for c in range(NC):
        for grp in range(NG):
            sb = int(lay.selfb[grp])
            idx16[c, grp, :, :sb // 16] = _pack_idx(idx_all[c, grp, :sb])

    # meta [c, grp, 128, MCMAX]: per block, interleaved (drel, norm) cols
    meta = np.zeros((NC, NG, P, lay.MCMAX), np.float32)
    pvec = np.arange(P)
    for B in range(NB):
        grp = B // GPG
        mb = int(lay.mstart[B])
        for j, o in enumerate(lay.tiles[B]):
            meta[:, grp, :, mb + 2 * j] = drel_all[:, grp, o + pvec]
            meta[:, grp, :, mb + 2 * j + 1] = norm_all[:, grp, o + pvec]

    # pooling one-hot [c, grp, 128, GPG*GPC]
    gcnt = np.bincount(batch, minlength=G).astype(np.float32)
    pool_oh = np.zeros((NC, NB, P, GPC), np.float32)
    loc = np.arange(N) - core_start[node_core]
    gl = batch - node_core * GPC
    pool_oh[node_core, loc // P, loc % P, gl] = 1.0 / np.maximum(gcnt[batch], 1.0)
    pool_oh = (pool_oh.reshape(NC, NG, GPG, P, GPC).transpose(0, 1, 3, 2, 4)
               .reshape(NC, NG, P, GPG * GPC).astype(ml_dtypes.bfloat16))

    # ---- edge encoder packing ----
    egraph = batch[src]
    ecnt = np.bincount(egraph, minlength=G).astype(np.float32)
    eorder = np.argsort(egraph, kind="stable")
    egr_s = egraph[eorder]
    gb = np.searchsorted(egr_s, np.arange(G + 1))
    chunks = []  # (core, graph, sel)
    for g_ in range(G):
        c = g_ // GPC
        s0, s1 = gb[g_], gb[g_ + 1]
        for o in range(s0, s1, ECH_CHUNK):
            chunks.append((c, g_, eorder[o:min(o + ECH_CHUNK, s1)]))
    ECH = max(sum(1 for ch in chunks if ch[0] == c) for c in range(NC))
    ECOLS = ((ECH + 7) // 8) * ECH_CHUNK
    ea_pack = np.zeros((NC, P, ECOLS), np.float32)
    chunk2graph = np.full((NC, ECH), -1, np.int64)
    chunk_pad = np.zeros((NC, ECH), np.int64)
    ci = [0] * NC
    for c, g_, sel in chunks:
        i = ci[c]
        ci[c] = i + 1
        chunk2graph[c, i] = g_
        chunk_pad[c, i] = ECH_CHUNK - len(sel)
        r0 = 16 * (i % 8)
        c0 = (i // 8) * ECH_CHUNK
        ea_pack[c, r0:r0 + DE, c0:c0 + len(sel)] = ea[sel].T
    ea_pack = ea_pack.astype(ml_dtypes.bfloat16)

    # folded weights
    def fold(Wm, bm, gm, btm, mm, vm):
        s = (gm / np.sqrt(vm + EPS)).astype(np.float32)
        return ((np.asarray(Wm, np.float32) * s[None, :]).astype(ml_dtypes.bfloat16),
                (((np.asarray(bm, np.float32) - mm) * s + btm)
                 .astype(ml_dtypes.bfloat16)))

    w1f, t1 = fold(W1, b1, g1, bt1, m1, v1)
    w2f, t2 = fold(W2, b2, g2, bt2, m2, v2)

    we1 = np.asarray(We1, np.float32).astype(ml_dtypes.bfloat16)
    wvar = np.zeros((P, 4 * D), ml_dtypes.bfloat16)
    for j in range(4):
        for h in range(2):
            wvar[64 * h + 16 * j:64 * h + 16 * (j + 1), j * D:(j + 1) * D] = we1

    common = {
        "x_table": x_table,
        "w1f": w1f, "t1r": t1[None, :],
        "w2f": w2f, "t2r": t2[None, :],
        "wvar": wvar,
        "be1c": np.asarray(be1, np.float32)[:, None],
        "iota128": np.tile(np.arange(P, dtype=ml_dtypes.bfloat16)[None, :], (P, 1)),
    }
    in_maps = []
    for c in range(NC):
        m = dict(common)
        m["idx16"] = idx16[c]
        m["meta"] = meta[c]
        m["pool_oh"] = pool_oh[c]
        m["ea_pack"] = ea_pack[c]
        in_maps.append(m)

    host = {
        "caps_key": tuple(caps_bw.reshape(-1).tolist()), "ECH": ECH,
        "chunk2graph": chunk2graph, "chunk_pad": chunk_pad, "ecnt": ecnt,
        "We2": np.asarray(We2, np.float32),
        "be2": np.asarray(be2, np.float32),
        "be1": np.asarray(be1, np.float32),
    }
    return in_maps, host


def _get_runner(caps_key, ECH):
    key = (caps_key, ECH)
    if _cache.get("key") != key:
        nc = _build_nc(caps_key, ECH)
        from concourse.bass_utils import run_bass_kernel_spmd
        _cache["key"] = key
        _cache["nc"] = nc
        _cache["run"] = lambda in_maps: run_bass_kernel_spmd(
            nc, in_maps, list(range(NC)))
    return _cache["nc"], _cache["run"]


def _assemble(res, host):
    """Combine per-core device outputs into the full [G, D] output."""
    ecnt = host["ecnt"]
    We2, be2 = host["We2"], host["be2"]
    c2g = host["chunk2graph"]
    relu_be1 = np.maximum(host["be1"], 0.0)
    out = np.zeros((G, D), np.float32)
    for c in range(NC):
        pool_t = res.results[c]["out_pool"]          # [D, GPC]
        out[c * GPC:(c + 1) * GPC, :] = pool_t.T
        eacc = res.results[c]["out_eacc"]            # [D, ECH]
        # zero-padded edge columns inside a chunk contribute relu(0 + be1)
        corr = np.asarray(eacc.T, np.float32) - np.outer(
            host["chunk_pad"][c].astype(np.float32), relu_be1)
        es = np.zeros((GPC, D), np.float32)
        valid = c2g[c] >= 0
        np.add.at(es, c2g[c][valid] - c * GPC, corr[valid])
        cg = ecnt[c * GPC:(c + 1) * GPC]
        es = np.where(cg[:, None] > 0,
                      es / np.maximum(cg, 1.0)[:, None] @ We2 + be2[None, :], 0.0)
        out[c * GPC:(c + 1) * GPC, :] += es
    return out


def kernel(x, edge_index, batch_idx, edge_attr, num_graphs,
           W1, b1, g1, bt1, m1, v1, W2, b2, g2, bt2, m2, v2,
           We1, be1, We2, be2):
    in_maps, host = _preprocess(x, edge_index, batch_idx, edge_attr,
                                W1, b1, g1, bt1, m1, v1, W2, b2, g2, bt2,
                                m2, v2, We1, be1, We2, be2)
    nc, run = _get_runner(host["caps_key"], host["ECH"])
    res = run(in_maps)
    return _assemble(res, host)
